# revision 19
# baseline (speedup 1.0000x reference)
"""CrossAttentionS2T (attn_all_frame=True) as a Bass/Tile kernel on 8 trn2 cores.

Strategy: data-parallel over batch B=8 -> one batch element per NeuronCore.
Host precomputes the positional adds, all transposes, and bf16 casts; the
device does the four GEMMs + attention in one software-pipelined pass:

  qT[f,qt]   = (0.125*Wq) @ qinT[:,qt]               (scale folded into Wq)
  kT         = Wk @ ssT ; v (natural, +ones col per head) = ssT.T @ Wv.T
  scores.T   = QK for head pairs packed into PE row groups (rows 0-63 /
               64-127) so two heads stream concurrently; scores ping-pong
               between two multi-bank PSUM tiles
  probs      = exp(scores), one ACT instruction per 2-3 PSUM banks
  [o.T;den]  = [v_h | 1]^T @ probs.T   (ones column => softmax denominator)
  o.T        = o.T * bcast(1/den)      (DVE reciprocal_approx_fast from SBUF
               + gpsimd partition_broadcast + DVE multiply)
  outT       = Wp @ o.T + pb -> DMA out as bf16 [768, 1568]; host transposes.

All GEMM operands are bf16 (PSUM accumulation is fp32); exp input is the
fp32 PSUM scores. Emission is dependency-topological to avoid head-of-line
blocking in the static per-engine queues: QK groups lead, AV matmuls follow
their exp, the normalization drain of pair p is deferred into pair p+1's
stream, and Q/O projection groups are spread through the pair loop as PE
filler.
"""

import math
import os
from contextlib import ExitStack

import numpy as np

import concourse.bass as bass
import concourse.mybir as mybir
import concourse.tile as tile
from concourse.bass import ds, ts

F32 = mybir.dt.float32
BF16 = mybir.dt.bfloat16
AF = mybir.ActivationFunctionType

# problem dims (hardcoded per contract)
B, SPEC, T = 8, 4, 8
AP_, VP, DIM = 196, 196, 768
NH, HD = 12, 64
SCALE = HD ** -0.5
NQ = VP * T          # 1568 q tokens per batch
NK = AP_ * SPEC      # 784 kv tokens per batch
DC = DIM // 128      # 6 contraction chunks
QT, NQT = 392, 4     # q-token tile (moving free dim)
KB, NKB = 112, 7     # k-token block (scores.T partition dim)
VW, NVW = 384, 2     # v feature tile for natural-layout V projection
NCORES = 8

# exp group structure per head-pair: 14 (head, j) blocks in groups that
# ping-pong between two PSUM score tiles (3 banks + 2 banks).
# seq s = 2*j + (0 for even head, 1 for odd head of the pair).
GRP_SIZES = (3, 2, 3, 2, 2, 2)
GRP_START = (0, 3, 5, 8, 10, 12)


def _grp_of(s):
    for g in range(len(GRP_SIZES) - 1, -1, -1):
        if s >= GRP_START[g]:
            return g, s - GRP_START[g]
    raise AssertionError


def _emit(ctx, tc, outs, ins):
    nc = tc.nc
    (qinT_d, ssT_d, wqT, wkT, wvT, wpT, qb2, kb2, pb2, vbb) = ins
    out_d = outs[0]
    dbg = outs[1:] if len(outs) > 1 else None

    const = ctx.enter_context(tc.tile_pool(name="const", bufs=1))
    qb_t = const.tile([128, DC], F32)
    kb_t = const.tile([128, DC], F32)
    pb_t = const.tile([128, DC], F32)
    vbb_t = const.tile([128, DIM], F32)
    nc.sync.dma_start(qb_t[:], qb2[:])
    nc.sync.dma_start(kb_t[:], kb2[:])
    nc.sync.dma_start(pb_t[:], pb2[:])
    nc.sync.dma_start(vbb_t[:], vbb[:])

    # persistent pool: weights for Q/out proj, K/V activations, qin/qT streams
    pers = ctx.enter_context(tc.tile_pool(name="pers", bufs=1))
    wq_t = [pers.tile([128, DIM], BF16, name=f"wq{c}", tag=f"wq{c}")
            for c in range(DC)]
    wp_t = [pers.tile([128, DIM], BF16, name=f"wp{c}", tag=f"wp{c}")
            for c in range(DC)]
    kTt = [pers.tile([128, NK], BF16, name=f"kT{c}", tag=f"kT{c}")
           for c in range(DC)]
    v_t = [pers.tile([KB, NH * (HD + 1)], BF16, name=f"v{j}", tag=f"v{j}")
           for j in range(NKB)]

    # PSUM: sA (3 banks) + sB (2) + o (2) + proj (1) = 8 banks
    psum = ctx.enter_context(tc.tile_pool(name="psum", bufs=1, space="PSUM"))

    def psum_tile(shape, name, tag, bufs=1, padded_shape=None):
        return psum.tile(shape, F32, name=name, tag=tag, bufs=bufs,
                         padded_shape=padded_shape)

    # ---- phase A: K/V projections from host-prepped ssT ----
    with tc.tile_pool(name="phA", bufs=1) as phA:
        wk_t = [phA.tile([128, DIM], BF16, name=f"wk{c}", tag=f"wk{c}")
                for c in range(DC)]
        wv_t = [phA.tile([128, DIM], BF16, name=f"wv{c}", tag=f"wv{c}")
                for c in range(DC)]
        sT = [phA.tile([128, NK], BF16, name=f"sT{c}", tag=f"sT{c}")
              for c in range(DC)]
        # wk + sT first (Kproj gate), striped across all 3 DMA queues;
        # wv next; wq/wp and qin stream in behind.
        qs = [nc.sync, nc.scalar, nc.gpsimd]
        for i in range(2 * DC):
            c = i // 2
            if i % 2 == 0:
                qs[i % 3].dma_start(wk_t[c][:], wkT[ts(c, 128), :])
            else:
                qs[i % 3].dma_start(sT[c][:], ssT_d[ts(c, 128), :])
        for c in range(DC):
            qs[c % 3].dma_start(wv_t[c][:], wvT[ts(c, 128), :])
        for c in range(DC):
            nc.sync.dma_start(wq_t[c][:], wqT[ts(c, 128), :])
            nc.gpsimd.dma_start(wp_t[c][:], wpT[ts(c, 128), :])

        # during phase A the attention banks are free: cycle 4 psum slots
        pha_slots = [("sA", [128, 3, 512], 1), ("sB", [128, 2, 512], 1),
                     ("o", [128, 512], 2), ("proj", [128, 512], 1)]
        slot_i = [0]

        def pha_psum():
            tag, shape, bufs = pha_slots[slot_i[0] % len(pha_slots)]
            slot_i[0] += 1
            t = psum_tile(shape, "psA", tag, bufs=bufs)
            return t[:, 0, :] if len(shape) == 3 else t

        # K projection, transposed output layout [kfeat, ktok]
        for f in range(DC):
            for kt in range(2):
                ps = pha_psum()
                for c in range(DC):
                    nc.tensor.matmul(
                        ps[0:128, 0:QT], wk_t[c][:, ts(f, 128)],
                        sT[c][:, ts(kt, QT)],
                        start=(c == 0), stop=(c == DC - 1))
                nc.vector.tensor_scalar_add(
                    kTt[f][:, ts(kt, QT)], ps[0:128, 0:QT], kb_t[:, ds(f, 1)])

        # V projection, natural layout [ktok, vfeat] bf16, +1s col per head
        for j in range(NKB):
            v3 = v_t[j].rearrange("p (h e) -> p h e", e=HD + 1)
            nc.vector.memset(v3[:, :, ds(HD, 1)], 1.0)
            for w in range(NVW):
                ps = pha_psum()
                for c in range(DC):
                    nc.tensor.matmul(
                        ps[0:KB, 0:VW], sT[c][:, ts(j, KB)],
                        wv_t[c][:, ts(w, VW)],
                        start=(c == 0), stop=(c == DC - 1))
                nc.vector.tensor_add(
                    v3[:, ds(w * 6, 6), 0:HD],
                    ps[0:KB, 0:VW].rearrange("p (h e) -> p h e", e=HD),
                    vbb_t[0:KB, ts(w, VW)].rearrange("p (h e) -> p h e", e=HD))
        if dbg is not None:
            for f in range(DC):
                nc.sync.dma_start(dbg[0][ts(f, 128), :], kTt[f][:])

    # ---- main pipeline ----
    main = ctx.enter_context(tc.tile_pool(name="main", bufs=1))

    def emit_qproj(f, qins, qTt):
        ps = psum_tile([128, 512], "ps_q", "proj")
        for c in range(DC):
            nc.tensor.matmul(
                ps[0:128, 0:QT], wq_t[c][:, ts(f, 128)], qins[c][:],
                start=(c == 0), stop=(c == DC - 1))
        qT_f = pers.tile([128, QT], BF16, name="qT", tag="qT", bufs=12)
        nc.vector.tensor_scalar_add(qT_f[:], ps[0:128, 0:QT],
                                    qb_t[:, ds(f, 1)])
        qTt.append(qT_f)

    def emit_qin_dma(qt):
        qins = []
        for c in range(DC):
            qin_c = pers.tile([128, QT], BF16, name="qin", tag="qin", bufs=12)
            nc.gpsimd.dma_start(qin_c[:], qinT_d[ts(c, 128), ts(qt, QT)])
            qins.append(qin_c)
        return qins

    def emit_oproj(qt, f, oT):
        ps = psum_tile([128, 512], "ps_o", "proj")
        for c in range(DC):
            nc.tensor.matmul(
                ps[0:128, 0:QT], wp_t[c][:, ts(f, 128)], oT[c][:],
                start=(c == 0), stop=(c == DC - 1))
        outT_f = main.tile([128, QT], BF16, name="outT", tag="outT", bufs=4)
        nc.vector.tensor_scalar_add(outT_f[:], ps[0:128, 0:QT],
                                    pb_t[:, ds(f, 1)])
        nc.sync.dma_start(out_d[ts(f, 128), ts(qt, QT)], outT_f[:])

    qins0 = emit_qin_dma(0)
    qins_nxt = emit_qin_dma(1)
    qT_cur, qT_nxt = [], []
    for f in range(DC):
        emit_qproj(f, qins0, qT_cur)

    pend_drain = []
    oT_prev, oT_cur = None, None
    for qt in range(NQT):
        oT_prev = oT_cur
        oT_cur = [main.tile([128, QT], BF16, name=f"oT{c}", tag=f"oT{c}",
                            bufs=2) for c in range(DC)]
        qT_nxt = []
        if qt >= 1 and qt + 1 < NQT:
            qins_nxt = emit_qin_dma(qt + 1)

        for ch in range(DC):
            probs = [None] * len(GRP_SIZES)
            o_ps_pair = [psum_tile([128, 512], "o_ps", "o", bufs=2)
                         for _ in range(2)]

            def emit_qk(g):
                gsz = GRP_SIZES[g]
                if g % 2 == 0:
                    st = psum_tile([128, gsz, 512], "s_psA", "sA",
                                   padded_shape=[128, 3, 512])
                else:
                    st = psum_tile([128, gsz, 512], "s_psB", "sB")
                for slot in range(gsz):
                    s = GRP_START[g] + slot
                    par, j = (s % 2) * HD, s // 2
                    nc.tensor.matmul(
                        st[0:KB, slot, 0:QT],
                        kTt[ch][ds(par, HD), ts(j, KB)],
                        qT_cur[ch][ds(par, HD), :],
                        start=True, stop=True)
                return st

            def emit_exp(g, st):
                gsz = GRP_SIZES[g]
                p_t = main.tile([KB, gsz, QT], BF16, name=f"pr{gsz}",
                                tag=f"pr{gsz}", bufs=4 if gsz == 3 else 8)
                nc.scalar.activation(p_t[0:KB, :, :],
                                     st[0:KB, 0:gsz, 0:QT], AF.Exp)
                probs[g] = p_t
                if dbg is not None and qt == 0 and ch == 0:
                    nc.sync.dma_start(dbg[2][:, ds(GRP_START[g], gsz), :],
                                      p_t[0:KB, :, :])

            def emit_av(g):
                for slot in range(GRP_SIZES[g]):
                    s = GRP_START[g] + slot
                    h_odd, j = s % 2, s // 2
                    h = 2 * ch + h_odd
                    nc.tensor.matmul(
                        o_ps_pair[h_odd][0:HD + 1, 0:QT],
                        v_t[j][:, ds(h * (HD + 1), HD + 1)],
                        probs[g][0:KB, slot, :],
                        start=(j == 0), stop=(j == NKB - 1),
                        skip_group_check=True)

            def emit_drain(oTd):
                for (ch2, h_odd2, o_ps2, r1_2) in pend_drain:
                    rb = main.tile([HD, QT], F32, name="rb", tag="rb",
                                   bufs=4)
                    nc.gpsimd.partition_broadcast(rb[:], r1_2[:])
                    nc.vector.tensor_mul(
                        oTd[ch2][ds(h_odd2 * HD, HD), :],
                        o_ps2[0:HD, 0:QT], rb[:])
                del pend_drain[:]

            st0 = emit_qk(0)
            st1 = emit_qk(1)
            emit_exp(0, st0)
            emit_exp(1, st1)
            if pend_drain:
                emit_drain(oT_cur if ch > 0 else oT_prev)
            emit_av(0)
            if oT_prev is not None:
                emit_oproj(qt - 1, ch, oT_prev)
            st2 = emit_qk(2)
            emit_exp(2, st2)
            emit_av(1)
            st3 = emit_qk(3)
            emit_exp(3, st3)
            emit_av(2)
            if qt + 1 < NQT:
                emit_qproj(ch, qins_nxt, qT_nxt)
            else:
                # dummy PE filler group: keeps the HAM activity monitor from
                # re-throttling the clock during the final q-tile (which has
                # no next-tile Q projection to compute). Result is discarded.
                ps = psum_tile([128, 512], "ps_dummy", "proj")
                for c in range(DC):
                    nc.tensor.matmul(
                        ps[0:128, 0:QT], kTt[c][:, 0:128],
                        kTt[c][:, 0:QT], start=(c == 0), stop=(c == DC - 1))
            st4 = emit_qk(4)
            emit_exp(4, st4)
            emit_av(3)
            st5 = emit_qk(5)
            emit_exp(5, st5)
            emit_av(4)
            emit_av(5)

            for h_odd in range(2):
                den_sb = main.tile([1, QT], F32, name="den_sb",
                                   tag="den", bufs=4)
                nc.vector.tensor_copy(den_sb[:],
                                      o_ps_pair[h_odd][ds(HD, 1), 0:QT])
                r1 = main.tile([1, QT], F32, name="r1", tag="r1", bufs=4)
                nc.vector.reciprocal_approx_fast(r1[:], den_sb[:])
                pend_drain.append((ch, h_odd, o_ps_pair[h_odd], r1))

        if dbg is not None and qt == 0:
            for f in range(DC):
                nc.sync.dma_start(dbg[1][ts(f, 128), :], qT_cur[f][:])
            for j in range(NKB):
                nc.sync.dma_start(dbg[4][ds(j * KB, KB), :], v_t[j][:])

        if qt + 1 < NQT:
            qT_cur = qT_nxt

    # epilogue: drain last pair + final out projection
    for (ch2, h_odd2, o_ps2, r1_2) in pend_drain:
        rb = main.tile([HD, QT], F32, name="rb", tag="rb", bufs=4)
        nc.gpsimd.partition_broadcast(rb[:], r1_2[:])
        nc.vector.tensor_mul(oT_cur[ch2][ds(h_odd2 * HD, HD), :],
                             o_ps2[0:HD, 0:QT], rb[:])
    del pend_drain[:]
    if dbg is not None:
        for c in range(DC):
            nc.sync.dma_start(dbg[3][ts(c, 128), :], oT_cur[c][:])
    for f in range(DC):
        emit_oproj(NQT - 1, f, oT_cur)


def build_program():
    from concourse import bacc
    from concourse.compiler_utils import get_compiler_flags, set_compiler_flags
    flags = [f.replace("--enable-ldw-opt=false", "--enable-ldw-opt=true")
             for f in get_compiler_flags()]
    set_compiler_flags(flags)
    nc = bacc.Bacc("TRN2", target_bir_lowering=False, debug=False,
                   num_devices=NCORES)

    def mk(name, shape, dt=BF16, out=False):
        return nc.dram_tensor(
            name, shape, dt,
            kind="ExternalOutput" if out else "ExternalInput").ap()

    ins = [
        mk("qinT", [DIM, NQ]), mk("ssT", [DIM, NK]),
        mk("wqT", [DIM, DIM]), mk("wkT", [DIM, DIM]),
        mk("wvT", [DIM, DIM]), mk("wpT", [DIM, DIM]),
        mk("qb2", [128, DC], F32), mk("kb2", [128, DC], F32),
        mk("pb2", [128, DC], F32), mk("vbb", [128, DIM], F32),
    ]
    outs = [mk("out", [DIM, NQ], out=True)]
    if os.environ.get("KDBG"):
        outs.append(mk("dbg_kT", [DIM, NK], out=True))
        outs.append(mk("dbg_qT", [DIM, QT], out=True))
        outs.append(mk("dbg_probs", [KB, 14, QT], out=True))
        outs.append(mk("dbg_oT", [DIM, QT], out=True))
        outs.append(mk("dbg_v", [NKB * KB, NH * (HD + 1)], out=True))
    with tile.TileContext(nc) as tc:
        with ExitStack() as ctx:
            _emit(ctx, tc, outs, ins)
    nc.compile()
    return nc


def host_prep(inputs):
    """Host-side layout marshalling: slice per core, add positional embeds,
    transpose to [feature, token], fold the attention scale into Wq
    (exact: 0.125 = 2^-3), cast GEMM operands to bf16."""
    import ml_dtypes
    bf16 = ml_dtypes.bfloat16
    f32 = np.float32
    g = {k: np.asarray(v, dtype=f32) for k, v in inputs.items()}
    t_pat = g["t_x"][1:]                      # (VP, B*T, D)
    s_x = g["s_x"]                            # (AP, B*SPEC, D)

    posq = (g["vmae_space_pos"][:, None, :] + g["vmae_temporal_pos"][None, :, :])
    posq = posq.reshape(NQ, DIM)
    poss = (g["clip_space_pos"][:, None, :] + g["clip_temporal_pos"][None, :, :])
    poss = poss.reshape(NK, DIM)

    wqT = np.ascontiguousarray((SCALE * g["Wq"]).T).astype(bf16)
    wkT = np.ascontiguousarray(g["Wkv"][:DIM].T).astype(bf16)
    wvT = np.ascontiguousarray(g["Wkv"][DIM:].T).astype(bf16)
    wpT = np.ascontiguousarray(g["Wproj"].T).astype(bf16)
    qb2 = np.ascontiguousarray((SCALE * g["q_bias"]).reshape(DC, 128).T)
    kb2 = np.ascontiguousarray(g["kv_bias"][:DIM].reshape(DC, 128).T)
    pb2 = np.ascontiguousarray(g["proj_bias"].reshape(DC, 128).T)
    vbb = np.ascontiguousarray(np.tile(g["kv_bias"][DIM:], (128, 1)))

    shared = dict(wqT=wqT, wkT=wkT, wvT=wvT, wpT=wpT,
                  qb2=qb2, kb2=kb2, pb2=pb2, vbb=vbb)
    in_maps = []
    for b in range(B):
        qin = t_pat[:, b * T:(b + 1) * T, :].reshape(NQ, DIM) + posq
        ss = s_x[:, b * SPEC:(b + 1) * SPEC, :].reshape(NK, DIM) + poss
        in_maps.append(dict(qinT=np.ascontiguousarray(qin.T).astype(bf16),
                            ssT=np.ascontiguousarray(ss.T).astype(bf16),
                            **shared))
    return in_maps


def host_finish(results, t_x):
    # per-core out is bf16 [DIM, NQ] feature-major; transpose on host
    o = np.stack([np.asarray(results[b]["out"], dtype=np.float32).T
                  for b in range(B)])                      # (B, NQ, D)
    o = o.reshape(B, VP, T, DIM).transpose(1, 0, 2, 3).reshape(VP, B * T, DIM)
    return np.concatenate([np.asarray(t_x, dtype=np.float32)[0:1], o], axis=0)


_NC = None


def kernel(**inputs):
    global _NC
    from concourse.bass_utils import run_bass_kernel_spmd
    if _NC is None:
        _NC = build_program()
    in_maps = host_prep(inputs)
    res = run_bass_kernel_spmd(_NC, in_maps, list(range(NCORES)))
    return host_finish(res.results, inputs["t_x"])


# revision 20
# speedup vs baseline: 1.0459x; 1.0459x over previous
"""CrossAttentionS2T (attn_all_frame=True) as a Bass/Tile kernel on 8 trn2 cores.

Strategy: data-parallel over batch B=8 -> one batch element per NeuronCore.
Host precomputes the positional adds, all transposes, and bf16 casts; the
device does the four GEMMs + attention in one software-pipelined pass:

  qT[f,qt]   = (0.125*Wq) @ qinT[:,qt]               (scale folded into Wq)
  kT         = Wk @ ssT ; v (natural, +ones col per head) = ssT.T @ Wv.T
  scores.T   = QK for head pairs packed into PE row groups (rows 0-63 /
               64-127) so two heads stream concurrently; scores ping-pong
               between two multi-bank PSUM tiles
  probs      = exp(scores), one ACT instruction per 2-3 PSUM banks
  [o.T;den]  = [v_h | 1]^T @ probs.T   (ones column => softmax denominator)
  o.T        = o.T * bcast(1/den)      (DVE reciprocal_approx_fast from SBUF
               + gpsimd partition_broadcast + DVE multiply)
  outT       = Wp @ o.T + pb -> DMA out as bf16 [768, 1568]; host transposes.

All GEMM operands are bf16 (PSUM accumulation is fp32); exp input is the
fp32 PSUM scores. Emission is dependency-topological to avoid head-of-line
blocking in the static per-engine queues: QK groups lead, AV matmuls follow
their exp, the normalization drain of pair p is deferred into pair p+1's
stream, and Q/O projection groups are spread through the pair loop as PE
filler.
"""

import math
import os
from contextlib import ExitStack

import numpy as np

import concourse.bass as bass
import concourse.mybir as mybir
import concourse.tile as tile
from concourse.bass import ds, ts

F32 = mybir.dt.float32
BF16 = mybir.dt.bfloat16
AF = mybir.ActivationFunctionType

# problem dims (hardcoded per contract)
B, SPEC, T = 8, 4, 8
AP_, VP, DIM = 196, 196, 768
NH, HD = 12, 64
SCALE = HD ** -0.5
NQ = VP * T          # 1568 q tokens per batch
NK = AP_ * SPEC      # 784 kv tokens per batch
DC = DIM // 128      # 6 contraction chunks
QT, NQT = 392, 4     # q-token tile (moving free dim)
KB, NKB = 112, 7     # k-token block (scores.T partition dim)
VW, NVW = 384, 2     # v feature tile for natural-layout V projection
NCORES = 8

# exp group structure per head-pair: 14 (head, j) blocks in groups that
# ping-pong between two PSUM score tiles (3 banks + 2 banks).
# seq s = 2*j + (0 for even head, 1 for odd head of the pair).
GRP_SIZES = (3, 2, 3, 2, 2, 2)
GRP_START = (0, 3, 5, 8, 10, 12)


def _grp_of(s):
    for g in range(len(GRP_SIZES) - 1, -1, -1):
        if s >= GRP_START[g]:
            return g, s - GRP_START[g]
    raise AssertionError


def _emit(ctx, tc, outs, ins):
    nc = tc.nc
    (qinT_d, ssT_d, wqT, wkT, wvT, wpT, qb2, kb2, pb2, vbb) = ins
    out_d = outs[0]
    dbg = outs[1:] if len(outs) > 1 else None

    const = ctx.enter_context(tc.tile_pool(name="const", bufs=1))
    qb_t = const.tile([128, DC], F32)
    kb_t = const.tile([128, DC], F32)
    pb_t = const.tile([128, DC], F32)
    vbb_t = const.tile([128, DIM], F32)
    nc.sync.dma_start(qb_t[:], qb2[:])
    nc.sync.dma_start(kb_t[:], kb2[:])
    nc.sync.dma_start(pb_t[:], pb2[:])
    nc.sync.dma_start(vbb_t[:], vbb[:])

    # persistent pool: weights for Q/out proj, K/V activations, qin/qT streams
    pers = ctx.enter_context(tc.tile_pool(name="pers", bufs=1))
    wq_t3 = pers.tile([128, DC, DIM], BF16, name="wq3", tag="wq3")
    wp_t3 = pers.tile([128, DC, DIM], BF16, name="wp3", tag="wp3")
    wq_t = [wq_t3[:, c, :] for c in range(DC)]
    wp_t = [wp_t3[:, c, :] for c in range(DC)]
    kTt = [pers.tile([128, NK], BF16, name=f"kT{c}", tag=f"kT{c}")
           for c in range(DC)]
    v_t = [pers.tile([KB, NH * (HD + 1)], BF16, name=f"v{j}", tag=f"v{j}")
           for j in range(NKB)]

    # PSUM: sA (3 banks) + sB (2) + o (2) + proj (1) = 8 banks
    psum = ctx.enter_context(tc.tile_pool(name="psum", bufs=1, space="PSUM"))

    def psum_tile(shape, name, tag, bufs=1, padded_shape=None):
        return psum.tile(shape, F32, name=name, tag=tag, bufs=bufs,
                         padded_shape=padded_shape)

    # ---- phase A: K/V projections from host-prepped ssT ----
    with tc.tile_pool(name="phA", bufs=1) as phA:
        wk_t3 = phA.tile([128, DC, DIM], BF16, name="wk3", tag="wk3")
        wv_t3 = phA.tile([128, DC, DIM], BF16, name="wv3", tag="wv3")
        sT3 = phA.tile([128, DC, NK], BF16, name="sT3", tag="sT3")
        wk_t = [wk_t3[:, c, :] for c in range(DC)]
        wv_t = [wv_t3[:, c, :] for c in range(DC)]
        sT = [sT3[:, c, :] for c in range(DC)]
        # sT + wv first (Vproj gate), wk next, wq/wp behind; one big-line
        # DMA per tensor, spread across the 3 DMA-capable queues.
        nc.scalar.dma_start(sT3[:], ssT_d[:])
        nc.gpsimd.dma_start(wv_t3[:], wvT[:])
        nc.sync.dma_start(wk_t3[:], wkT[:])
        nc.sync.dma_start(wq_t3[:], wqT[:])
        nc.gpsimd.dma_start(wp_t3[:], wpT[:])

        # during phase A the attention banks are free: cycle 4 psum slots
        pha_slots = [("sA", [128, 3, 512], 1), ("sB", [128, 2, 512], 1),
                     ("o", [128, 512], 2), ("proj", [128, 512], 1)]
        slot_i = [0]

        def pha_psum():
            tag, shape, bufs = pha_slots[slot_i[0] % len(pha_slots)]
            slot_i[0] += 1
            t = psum_tile(shape, "psA", tag, bufs=bufs)
            return t[:, 0, :] if len(shape) == 3 else t

        # V projection, natural layout [ktok, vfeat] bf16, +1s col per head
        for j in range(NKB):
            v3 = v_t[j].rearrange("p (h e) -> p h e", e=HD + 1)
            nc.vector.memset(v3[:, :, ds(HD, 1)], 1.0)
            for w in range(NVW):
                ps = pha_psum()
                for c in range(DC):
                    nc.tensor.matmul(
                        ps[0:KB, 0:VW], sT[c][:, ts(j, KB)],
                        wv_t[c][:, ts(w, VW)],
                        start=(c == 0), stop=(c == DC - 1))
                nc.vector.tensor_add(
                    v3[:, ds(w * 6, 6), 0:HD],
                    ps[0:KB, 0:VW].rearrange("p (h e) -> p h e", e=HD),
                    vbb_t[0:KB, ts(w, VW)].rearrange("p (h e) -> p h e", e=HD))

        # K projection, transposed output layout [kfeat, ktok]
        for f in range(DC):
            for kt in range(2):
                ps = pha_psum()
                for c in range(DC):
                    nc.tensor.matmul(
                        ps[0:128, 0:QT], wk_t[c][:, ts(f, 128)],
                        sT[c][:, ts(kt, QT)],
                        start=(c == 0), stop=(c == DC - 1))
                nc.vector.tensor_scalar_add(
                    kTt[f][:, ts(kt, QT)], ps[0:128, 0:QT], kb_t[:, ds(f, 1)])
        if dbg is not None:
            for f in range(DC):
                nc.sync.dma_start(dbg[0][ts(f, 128), :], kTt[f][:])

    # ---- main pipeline ----
    main = ctx.enter_context(tc.tile_pool(name="main", bufs=1))

    def emit_qproj(f, qins, qTt):
        ps = psum_tile([128, 512], "ps_q", "proj")
        for c in range(DC):
            nc.tensor.matmul(
                ps[0:128, 0:QT], wq_t[c][:, ts(f, 128)], qins[c][:],
                start=(c == 0), stop=(c == DC - 1))
        qT_f = pers.tile([128, QT], BF16, name="qT", tag="qT", bufs=12)
        nc.vector.tensor_scalar_add(qT_f[:], ps[0:128, 0:QT],
                                    qb_t[:, ds(f, 1)])
        qTt.append(qT_f)

    def emit_qin_dma(qt):
        qin3 = pers.tile([128, DC, QT], BF16, name="qin", tag="qin", bufs=2)
        nc.gpsimd.dma_start(qin3[:], qinT_d[qt, :, :, :])
        return [qin3[:, c, :] for c in range(DC)]

    outT_state = {}

    def emit_oproj(qt, f, oT):
        ps = psum_tile([128, 512], "ps_o", "proj")
        for c in range(DC):
            nc.tensor.matmul(
                ps[0:128, 0:QT], wp_t[c][:, ts(f, 128)], oT[c][:],
                start=(c == 0), stop=(c == DC - 1))
        if f == 0:
            outT_state[qt] = main.tile([128, DC, QT], BF16, name="outT",
                                       tag="outT", bufs=2)
        outT3 = outT_state[qt]
        nc.vector.tensor_scalar_add(outT3[:, f, :], ps[0:128, 0:QT],
                                    pb_t[:, ds(f, 1)])
        if f == DC - 1:
            nc.sync.dma_start(out_d[qt, :, :, :], outT3[:])

    qins0 = emit_qin_dma(0)
    qins_nxt = emit_qin_dma(1)
    qT_cur, qT_nxt = [], []
    for f in range(DC):
        emit_qproj(f, qins0, qT_cur)

    pend_drain = []
    oT_prev, oT_cur = None, None
    for qt in range(NQT):
        oT_prev = oT_cur
        oT_cur = [main.tile([128, QT], BF16, name=f"oT{c}", tag=f"oT{c}",
                            bufs=2) for c in range(DC)]
        qT_nxt = []
        if qt >= 1 and qt + 1 < NQT:
            qins_nxt = emit_qin_dma(qt + 1)

        for ch in range(DC):
            probs = [None] * len(GRP_SIZES)
            o_ps_pair = [psum_tile([128, 512], "o_ps", "o", bufs=2)
                         for _ in range(2)]

            def emit_qk(g):
                gsz = GRP_SIZES[g]
                if g % 2 == 0:
                    st = psum_tile([128, gsz, 512], "s_psA", "sA",
                                   padded_shape=[128, 3, 512])
                else:
                    st = psum_tile([128, gsz, 512], "s_psB", "sB")
                for slot in range(gsz):
                    s = GRP_START[g] + slot
                    par, j = (s % 2) * HD, s // 2
                    nc.tensor.matmul(
                        st[0:KB, slot, 0:QT],
                        kTt[ch][ds(par, HD), ts(j, KB)],
                        qT_cur[ch][ds(par, HD), :],
                        start=True, stop=True)
                return st

            def emit_exp(g, st):
                gsz = GRP_SIZES[g]
                p_t = main.tile([KB, gsz, QT], BF16, name=f"pr{gsz}",
                                tag=f"pr{gsz}", bufs=4 if gsz == 3 else 8)
                nc.scalar.activation(p_t[0:KB, :, :],
                                     st[0:KB, 0:gsz, 0:QT], AF.Exp)
                probs[g] = p_t
                if dbg is not None and qt == 0 and ch == 0:
                    nc.sync.dma_start(dbg[2][:, ds(GRP_START[g], gsz), :],
                                      p_t[0:KB, :, :])

            def emit_av(g):
                for slot in range(GRP_SIZES[g]):
                    s = GRP_START[g] + slot
                    h_odd, j = s % 2, s // 2
                    h = 2 * ch + h_odd
                    nc.tensor.matmul(
                        o_ps_pair[h_odd][0:HD + 1, 0:QT],
                        v_t[j][:, ds(h * (HD + 1), HD + 1)],
                        probs[g][0:KB, slot, :],
                        start=(j == 0), stop=(j == NKB - 1),
                        skip_group_check=True)

            def emit_drain(oTd):
                for (ch2, h_odd2, o_ps2, r1_2) in pend_drain:
                    rb = main.tile([HD, QT], F32, name="rb", tag="rb",
                                   bufs=4)
                    nc.gpsimd.partition_broadcast(rb[:], r1_2[:])
                    nc.vector.tensor_mul(
                        oTd[ch2][ds(h_odd2 * HD, HD), :],
                        o_ps2[0:HD, 0:QT], rb[:])
                del pend_drain[:]

            st0 = emit_qk(0)
            st1 = emit_qk(1)
            emit_exp(0, st0)
            emit_exp(1, st1)
            if pend_drain:
                emit_drain(oT_cur if ch > 0 else oT_prev)
            emit_av(0)
            if oT_prev is not None:
                emit_oproj(qt - 1, ch, oT_prev)
            st2 = emit_qk(2)
            emit_exp(2, st2)
            emit_av(1)
            st3 = emit_qk(3)
            emit_exp(3, st3)
            emit_av(2)
            if qt + 1 < NQT:
                emit_qproj(ch, qins_nxt, qT_nxt)
            st4 = emit_qk(4)
            emit_exp(4, st4)
            emit_av(3)
            st5 = emit_qk(5)
            emit_exp(5, st5)
            emit_av(4)
            emit_av(5)

            for h_odd in range(2):
                den_sb = main.tile([1, QT], F32, name="den_sb",
                                   tag="den", bufs=4)
                nc.vector.tensor_copy(den_sb[:],
                                      o_ps_pair[h_odd][ds(HD, 1), 0:QT])
                r1 = main.tile([1, QT], F32, name="r1", tag="r1", bufs=4)
                nc.vector.reciprocal_approx_fast(r1[:], den_sb[:])
                pend_drain.append((ch, h_odd, o_ps_pair[h_odd], r1))

        if dbg is not None and qt == 0:
            for f in range(DC):
                nc.sync.dma_start(dbg[1][ts(f, 128), :], qT_cur[f][:])
            for j in range(NKB):
                nc.sync.dma_start(dbg[4][ds(j * KB, KB), :], v_t[j][:])

        if qt + 1 < NQT:
            qT_cur = qT_nxt

    # epilogue: drain last pair + final out projection
    for (ch2, h_odd2, o_ps2, r1_2) in pend_drain:
        rb = main.tile([HD, QT], F32, name="rb", tag="rb", bufs=4)
        nc.gpsimd.partition_broadcast(rb[:], r1_2[:])
        nc.vector.tensor_mul(oT_cur[ch2][ds(h_odd2 * HD, HD), :],
                             o_ps2[0:HD, 0:QT], rb[:])
    del pend_drain[:]
    if dbg is not None:
        for c in range(DC):
            nc.sync.dma_start(dbg[3][ts(c, 128), :], oT_cur[c][:])
    for f in range(DC):
        emit_oproj(NQT - 1, f, oT_cur)


def build_program():
    from concourse import bacc
    from concourse.compiler_utils import get_compiler_flags, set_compiler_flags
    flags = [f.replace("--enable-ldw-opt=false", "--enable-ldw-opt=true")
             for f in get_compiler_flags()]
    set_compiler_flags(flags)
    nc = bacc.Bacc("TRN2", target_bir_lowering=False, debug=False,
                   num_devices=NCORES)

    def mk(name, shape, dt=BF16, out=False):
        return nc.dram_tensor(
            name, shape, dt,
            kind="ExternalOutput" if out else "ExternalInput").ap()

    ins = [
        mk("qinT", [NQT, 128, DC, QT]), mk("ssT", [128, DC, NK]),
        mk("wqT", [128, DC, DIM]), mk("wkT", [128, DC, DIM]),
        mk("wvT", [128, DC, DIM]), mk("wpT", [128, DC, DIM]),
        mk("qb2", [128, DC], F32), mk("kb2", [128, DC], F32),
        mk("pb2", [128, DC], F32), mk("vbb", [128, DIM], F32),
    ]
    outs = [mk("out", [NQT, 128, DC, QT], out=True)]
    if os.environ.get("KDBG"):
        outs.append(mk("dbg_kT", [DIM, NK], out=True))
        outs.append(mk("dbg_qT", [DIM, QT], out=True))
        outs.append(mk("dbg_probs", [KB, 14, QT], out=True))
        outs.append(mk("dbg_oT", [DIM, QT], out=True))
        outs.append(mk("dbg_v", [NKB * KB, NH * (HD + 1)], out=True))
    with tile.TileContext(nc) as tc:
        with ExitStack() as ctx:
            _emit(ctx, tc, outs, ins)
    nc.compile()
    return nc


def host_prep(inputs):
    """Host-side layout marshalling: slice per core, add positional embeds,
    transpose to [feature, token], fold the attention scale into Wq
    (exact: 0.125 = 2^-3), cast GEMM operands to bf16."""
    import ml_dtypes
    bf16 = ml_dtypes.bfloat16
    f32 = np.float32
    g = {k: np.asarray(v, dtype=f32) for k, v in inputs.items()}
    t_pat = g["t_x"][1:]                      # (VP, B*T, D)
    s_x = g["s_x"]                            # (AP, B*SPEC, D)

    posq = (g["vmae_space_pos"][:, None, :] + g["vmae_temporal_pos"][None, :, :])
    posq = posq.reshape(NQ, DIM)
    poss = (g["clip_space_pos"][:, None, :] + g["clip_temporal_pos"][None, :, :])
    poss = poss.reshape(NK, DIM)

    def packw(w):   # [d_in, f] -> [128, DC, f] big-line layout
        return np.ascontiguousarray(
            w.reshape(DC, 128, DIM).transpose(1, 0, 2)).astype(bf16)

    wqT = packw((SCALE * g["Wq"]).T)
    wkT = packw(g["Wkv"][:DIM].T)
    wvT = packw(g["Wkv"][DIM:].T)
    wpT = packw(g["Wproj"].T)
    qb2 = np.ascontiguousarray((SCALE * g["q_bias"]).reshape(DC, 128).T)
    kb2 = np.ascontiguousarray(g["kv_bias"][:DIM].reshape(DC, 128).T)
    pb2 = np.ascontiguousarray(g["proj_bias"].reshape(DC, 128).T)
    vbb = np.ascontiguousarray(np.tile(g["kv_bias"][DIM:], (128, 1)))

    shared = dict(wqT=wqT, wkT=wkT, wvT=wvT, wpT=wpT,
                  qb2=qb2, kb2=kb2, pb2=pb2, vbb=vbb)
    in_maps = []
    for b in range(B):
        qin = t_pat[:, b * T:(b + 1) * T, :].reshape(NQ, DIM) + posq
        ss = s_x[:, b * SPEC:(b + 1) * SPEC, :].reshape(NK, DIM) + poss
        # qin.T [D, NQ] -> [NQT, 128, DC, QT]; ss.T [D, NK] -> [128, DC, NK]
        qp = qin.T.reshape(DC, 128, NQT, QT).transpose(2, 1, 0, 3)
        sp = ss.T.reshape(DC, 128, NK).transpose(1, 0, 2)
        in_maps.append(dict(qinT=np.ascontiguousarray(qp).astype(bf16),
                            ssT=np.ascontiguousarray(sp).astype(bf16),
                            **shared))
    return in_maps


def host_finish(results, t_x):
    # per-core out is bf16 [NQT, 128, DC, QT]; unpack + transpose on host
    def unpack(o):   # -> [NQ, DIM]
        return o.transpose(0, 3, 2, 1).reshape(NQ, DIM)
    o = np.stack([unpack(np.asarray(results[b]["out"], dtype=np.float32))
                  for b in range(B)])                      # (B, NQ, D)
    o = o.reshape(B, VP, T, DIM).transpose(1, 0, 2, 3).reshape(VP, B * T, DIM)
    return np.concatenate([np.asarray(t_x, dtype=np.float32)[0:1], o], axis=0)


_NC = None


def kernel(**inputs):
    global _NC
    from concourse.bass_utils import run_bass_kernel_spmd
    if _NC is None:
        _NC = build_program()
    in_maps = host_prep(inputs)
    res = run_bass_kernel_spmd(_NC, in_maps, list(range(NCORES)))
    return host_finish(res.results, inputs["t_x"])
